# revision 10
# baseline (speedup 1.0000x reference)
"""Trainium2 distributed kernel for nn_AttentionFusion — v2.

Channel-major temporal attention, fp8 DoubleRow matmuls, PE score
reduction via block-ones matmuls, spatial mask folded into score PSUM,
direct strided-AP windows (no head-regroup scatter).

Per core: 2x4 grid of 50x25 blocks + 2-px halo (54x29 local = 1566 px,
padded to 13 chunks of 128).

Phase 1 (13 chunks): kT/qT ch-major + vPM pixel-major via fp8 DoubleRow;
scores = blockones matmuls over DVE products; softmax pixel-major;
ctx = p-weighted v (Pool); spatial projections from fp8 ctxT.
Phase 2 (25 chunks of 10x5 queries): score matmuls straight off kq_all
strided window APs; NEG mask added into PSUM by matmul; exp w/ scale.
Phase 3 (10 chunks): output projection; host rescales.
"""

import math
import os
import sys

import numpy as np

sys.path.insert(0, "/opt/trn_rl_repo")

import ml_dtypes  # noqa: E402

import concourse.bass as bass  # noqa: E402
import concourse.bacc as bacc  # noqa: E402
import concourse.mybir as mybir  # noqa: E402
import concourse.tile as tile  # noqa: E402

F32 = mybir.dt.float32
FP8 = mybir.dt.float8e4
BF16 = mybir.dt.bfloat16
AX = mybir.AxisListType
ALU = mybir.AluOpType
ACTF = mybir.ActivationFunctionType
DR = mybir.MatmulPerfMode.DoubleRow

# Problem constants
N_FULL = 10000
GRID = 100
T = 5
C = 256
NH = 8
DK = 32
CORES = 8
CR, CC_ = 2, 4             # core grid 2 x 4
BR, BC = 50, 25            # block rows/cols per core
NLOC = BR * BC             # 1250 own pixels per core
HR, HC = BR + 4, BC + 4    # 54 x 29 local region (with halo)
NH_PIX = HR * HC           # 1566
NP = 128
G = (NH_PIX + NP - 1) // NP        # 13 chunks
NPAD = G * NP                      # 1664
GO = 10                            # output-projection chunks
NPO = NLOC // GO                   # 125
SQR, SQC = 10, 5                   # query block 10 x 5
NS_R, NS_C = BR // SQR, BC // SQC  # 5 x 5 = 25 spatial chunks
NS = NS_R * NS_C
NQ = SQR * SQC                     # 50
WR, WC = SQR + 4, SQC + 4          # 14 x 9 window
NW = WR * WC                       # 126
NEGM = -1e9

_CACHE = {}


def _bf16(a):
    return np.asarray(a, dtype=ml_dtypes.bfloat16)


def _fp8(a):
    return np.asarray(a, dtype=ml_dtypes.float8_e4m3)


def _pow2_scale(w, target=8.0):
    rms = float(np.sqrt(np.mean(np.asarray(w, np.float64) ** 2)))
    s = 2.0 ** round(math.log2(target / max(rms, 1e-30)))
    assert float(np.abs(w).max()) * s < 350.0, "fp8 overflow risk"
    return s


def _build_graph():
    nc = bacc.Bacc(
        "TRN2",
        target_bir_lowering=False,
        debug=False,
        enable_asserts=False,
        num_devices=CORES,
    )

    # ---------------- I/O ----------------
    x_d = nc.dram_tensor("x", [G, 128, 2 * T * NP], FP8, kind="ExternalInput")
    wt_d = nc.dram_tensor("wt", [2, 128, 3 * C], FP8, kind="ExternalInput")
    we_d = nc.dram_tensor("we", [2, 128, 3 * C], FP8, kind="ExternalInput")
    wo_d = nc.dram_tensor("wo", [2, 128, C], FP8, kind="ExternalInput")
    bqt_d = nc.dram_tensor("bqt", [1, C], BF16, kind="ExternalInput")
    bqe_d = nc.dram_tensor("bqe", [1, C], BF16, kind="ExternalInput")
    boe_d = nc.dram_tensor("boe", [1, C], BF16, kind="ExternalInput")
    bo4_d = nc.dram_tensor("bo4", [128, 4], BF16, kind="ExternalInput")
    ident_d = nc.dram_tensor("ident", [128, 128], BF16, kind="ExternalInput")
    ones1_d = nc.dram_tensor("ones1", [1, 128], BF16, kind="ExternalInput")
    masks_d = nc.dram_tensor("masks", [NW, NS * NQ], BF16, kind="ExternalInput")
    scal_d = nc.dram_tensor("scal", [128, 2], F32, kind="ExternalInput")
    out_d = nc.dram_tensor("out", [NLOC, C], BF16, kind="ExternalOutput")

    with tile.TileContext(nc) as tc:
        with (
            tc.tile_pool(name="const", bufs=1) as cpool,
            tc.tile_pool(name="dram", bufs=1, space="DRAM") as dpool,
            tc.tile_pool(name="sb", bufs=2) as sb,
            tc.tile_pool(name="kp", bufs=2, space="PSUM") as pk,
            tc.tile_pool(name="qs", bufs=1, space="PSUM") as pq,
            tc.tile_pool(name="vp", bufs=2, space="PSUM") as pv,
            tc.tile_pool(name="ep", bufs=1, space="PSUM") as pe,
            tc.tile_pool(name="scx", bufs=2, space="PSUM") as px2,
            tc.tile_pool(name="vwp", bufs=6) as vwp,
        ):
            v_dram = dpool.tile([NPAD, C], BF16, tag="v_dram", name="v_dram")

            # ---------- constants ----------
            # per-projection weight tiles: k-tile pitch must equal the
            # moving free size for DoubleRow (contiguous [2, 256] runs)
            wt3, we3 = [], []
            for j in range(3):
                t_ = cpool.tile([128, 2, C], FP8, tag=f"wt{j}")
                nc.sync.dma_start(
                    t_[:], wt_d.ap()[:, :, C * j : C * (j + 1)].rearrange(
                        "a p c -> p a c")
                )
                wt3.append(t_)
                e_ = cpool.tile([128, 2, C], FP8, tag=f"we{j}")
                nc.sync.dma_start(
                    e_[:], we_d.ap()[:, :, C * j : C * (j + 1)].rearrange(
                        "a p c -> p a c")
                )
                we3.append(e_)
            wtk, wtv, wtq = wt3
            wek, weq, wev = we3
            wo = cpool.tile([128, 2, C], FP8, tag="wo")
            nc.sync.dma_start(wo[:], wo_d.ap().rearrange("a p c -> p a c"))
            bqt = cpool.tile([1, C], BF16, tag="bqt")
            nc.sync.dma_start(bqt[:], bqt_d.ap())
            bqe = cpool.tile([1, C], BF16, tag="bqe")
            nc.sync.dma_start(bqe[:], bqe_d.ap())
            boe = cpool.tile([1, C], BF16, tag="boe")
            nc.sync.dma_start(boe[:], boe_d.ap())
            bo4 = cpool.tile([128, 4], BF16, tag="bo4")
            nc.sync.dma_start(bo4[:], bo4_d.ap())
            ident = cpool.tile([128, 128], BF16, tag="ident")
            nc.sync.dma_start(ident[:], ident_d.ap())
            ones1 = cpool.tile([1, 128], BF16, tag="ones1")
            nc.sync.dma_start(ones1[:], ones1_d.ap())
            masks = cpool.tile([128, NS, NQ], BF16, tag="masks")
            nc.sync.dma_start(
                masks[0:NW, :, :], masks_d.ap().rearrange("w (s q) -> w s q", s=NS)
            )
            onesw = cpool.tile([128, 1], BF16, tag="onesw")
            nc.vector.memset(onesw[:], 1.0)
            # runtime f32 scales (per-partition replicated):
            # col 0 = temporal exp scale, col 1 = spatial exp scale
            scal = cpool.tile([128, 2], F32, tag="scal")
            nc.sync.dma_start(scal[:], scal_d.ap())

            kq_all = cpool.tile([128, 2, 2, NPAD], BF16, tag="kq_all")
            cT_all = cpool.tile([128, 2, NLOC], BF16, tag="cT_all")

            kqv = kq_all[:, :, :, 0:NH_PIX].rearrange(
                "p a b (r c) -> p a b r c", r=HR
            )
            vdv = v_dram[0:NH_PIX, :].rearrange("(r c) x -> r c x", r=HR)
            cTv = cT_all[:].rearrange("p a (r c) -> p a r c", r=BR)

            PH = os.environ.get("KERNEL_PHASES", "123")
            H = {}   # per-chunk tile handles passed from stage A to stage B

            def emit_A1(g):
                """temporal projections + prods for chunk g"""
                xg = sb.tile([128, 2, T, NP], FP8, tag="xg")
                nc.sync.dma_start(
                    xg[:], x_d.ap()[g].rearrange("p (a t n) -> p a t n", a=2, t=T)
                )
                # qT ch-major + bias; shares its PSUM bank with the score
                # accumulator s (disjoint byte ranges)
                qs_t = pq.tile([128, 296], F32, tag="qs", name=f"qs{g}")
                qp = qs_t[:, 0:256].rearrange("p (a n) -> p a n", a=2)
                st = qs_t[:, 256 : 256 + T * NH].rearrange(
                    "p (t h) -> p t h", t=T
                )
                for gq in range(2):
                    for cc in range(2):
                        nc.tensor.matmul(
                            qp[:, gq, :],
                            wtq[:, cc, 128 * gq : 128 * gq + 128],
                            xg[:, cc, T - 1, :],
                            start=(cc == 0), stop=False,
                            skip_group_check=True,
                        )
                    nc.tensor.matmul(
                        qp[:, gq, :],
                        bqt[0:1, 128 * gq : 128 * gq + 128],
                        ones1[0:1, 0:NP],
                        start=False, stop=True, skip_group_check=True,
                    )
                # hw: a DVE op may read at most ONE input from PSUM, so qT
                # moves to SBUF before the prod muls
                q_sb = sb.tile([128, 2, NP], BF16, tag="q_sb")
                if g % 2 == 0:
                    nc.scalar.copy(q_sb[:], qp[:])
                else:
                    nc.vector.tensor_copy(q_sb[:], qp[:])
                # kT ch-major in t-pairs; prod muls consume them right away.
                # All prods run before any s-matmul (whose start=True
                # pending-zeroes the whole shared bank in the sim, so qp must
                # be fully consumed first).
                prods = []
                for (t0, nt) in ((0, 2), (2, 2), (4, 1)):
                    kp_t = pk.tile([128, 2, 2, NP], F32, tag="kp",
                                   name=f"k{g}_{t0}")
                    for gk in range(2):
                        for cc in range(2):
                            nc.tensor.matmul(
                                kp_t[:, gk, 0:nt, :],
                                wtk[:, cc, 128 * gk : 128 * gk + 128],
                                xg[:, cc, t0 : t0 + nt, :],
                                start=(cc == 0), stop=(cc == 1),
                            )
                    prod = sb.tile([128, 2, 2, NP], BF16, tag=f"prod{t0}",
                                   name=f"prod{g}_{t0}")
                    nc.vector.tensor_mul(
                        prod[:, :, 0:nt, :],
                        kp_t[:, :, 0:nt, :],
                        q_sb[:].unsqueeze(2).broadcast_to((128, 2, nt, NP)),
                    )
                    prods.append((t0, nt, prod))
                # vPM pixel-major [px, t, (d h)] in t-pairs
                v_sb = sb.tile([128, T, C], BF16, tag="v_sb")
                for (t0, nt) in ((0, 2), (2, 2), (4, 1)):
                    vp_t = pv.tile([128, 512], F32, tag="vp",
                                   name=f"v{g}_{t0}")[:, 0 : nt * C].rearrange(
                        "p (a c) -> p a c", a=nt
                    )
                    for dt_ in range(nt):
                        for cc in range(2):
                            nc.tensor.matmul(
                                vp_t[:, dt_, :],
                                xg[:, cc, t0 + dt_, :],
                                wtv[:, cc, :],
                                start=(cc == 0), stop=(cc == 1),
                            )
                    nc.scalar.copy(v_sb[:, t0 : t0 + nt, :], vp_t[:])
                H[g] = {"prods": prods, "st": st, "v_sb": v_sb}

            def emit_A2(g):
                """scores + softmax for chunk g"""
                st = H[g]["st"]
                for (t0, nt, prod) in H[g].pop("prods"):
                    for gk in range(2):
                        for dt_ in range(nt):
                            nc.tensor.matmul(
                                st[:, t0 + dt_, 4 * gk : 4 * gk + 4],
                                prod[:, gk, dt_, :],
                                bo4[:, :],
                                start=True, stop=True, skip_group_check=True,
                            )
                es = sb.tile([128, T, NH], BF16, tag="es")
                nc.scalar.activation(es[:], st[:], ACTF.Exp,
                                     scale=scal[:, 0:1])
                tsum = sb.tile([128, NH], F32, tag="tsum")
                nc.vector.tensor_reduce(
                    tsum[:], es[:].rearrange("p t h -> p h t"), axis=AX.X,
                    op=ALU.add,
                )
                rinv = sb.tile([128, NH], F32, tag="rinv")
                nc.vector.reciprocal(rinv[:], tsum[:])
                p_t = sb.tile([128, T, NH], BF16, tag="p_t")
                nc.gpsimd.tensor_mul(
                    p_t[:], es[:], rinv[:].unsqueeze(1).broadcast_to((128, T, NH))
                )
                H[g]["p_t"] = p_t

            def emit_B1(g):
                """ctx accumulation (Pool) for chunk g"""
                p_t, v_sb = H[g]["p_t"], H[g]["v_sb"]
                wv = sb.tile([128, T, DK, NH], BF16, tag="wv")
                nc.gpsimd.tensor_mul(
                    wv[:],
                    v_sb[:].rearrange("p t (d h) -> p t d h", d=DK),
                    p_t[:].unsqueeze(2).broadcast_to((128, T, DK, NH)),
                )
                c1 = sb.tile([128, 2, C], BF16, tag="c1")
                nc.gpsimd.tensor_add(
                    c1[:].rearrange("p a (d h) -> p a d h", d=DK),
                    wv[:, 0:2], wv[:, 2:4],
                )
                c2 = sb.tile([128, C], BF16, tag="c2")
                nc.gpsimd.tensor_add(c2[:], c1[:, 0, :], c1[:, 1, :])
                ctx = sb.tile([128, C], BF16, tag="ctx")
                nc.gpsimd.tensor_add(
                    ctx[:].rearrange("p (d h) -> p d h", d=DK),
                    c2[:].rearrange("p (d h) -> p d h", d=DK),
                    wv[:, 4],
                )
                # ctxT via DMA transpose (no PSUM); bf16 moving operand
                # costs the same as fp8 without DoubleRow
                ctxT = sb.tile([128, 2, NP], BF16, tag="ctxT")
                nc.sync.dma_start_transpose(ctxT[:], ctx[:])
                H[g]["ctxT"] = ctxT

            def emit_B2(g):
                """spatial projections for chunk g"""
                ctxT = H[g]["ctxT"]

                # spatial projections: kqE [128, {k,q}, grp, px]
                kq_t = pe.tile([128, 512], F32, tag="ep", name=f"kq{g}")
                kqE = kq_t[:].rearrange("p (a b n) -> p a b n", a=2, b=2)
                last = g == G - 1   # chunk 12 has no query pixels
                for gk in range(2):
                    for cc in range(2):
                        nc.tensor.matmul(
                            kqE[:, 0, gk, :],
                            wek[:, cc, 128 * gk : 128 * gk + 128],
                            ctxT[:, cc, :],
                            start=(cc == 0), stop=(cc == 1),
                        )
                if not last:
                    for gk in range(2):
                        for cc in range(2):
                            nc.tensor.matmul(
                                kqE[:, 1, gk, :],
                                weq[:, cc, 128 * gk : 128 * gk + 128],
                                ctxT[:, cc, :],
                                start=(cc == 0), stop=False,
                                skip_group_check=True,
                            )
                        nc.tensor.matmul(
                            kqE[:, 1, gk, :],
                            bqe[0:1, 128 * gk : 128 * gk + 128],
                            ones1[0:1, 0:NP],
                            start=False, stop=True, skip_group_check=True,
                        )
                nkq = 1 if last else 2
                dst = kq_all[:, 0:nkq, :, g * NP : (g + 1) * NP]
                if g % 2 == 0:
                    nc.scalar.copy(dst, kqE[:, 0:nkq, :, :])
                else:
                    nc.vector.tensor_copy(dst, kqE[:, 0:nkq, :, :])

                # spatial v pixel-major
                vE = pv.tile([128, 512], F32, tag="vp",
                             name=f"vE{g}")[:, 0:C]
                for cc in range(2):
                    nc.tensor.matmul(
                        vE[:],
                        ctxT[:, cc, :],
                        wev[:, cc, :],
                        start=(cc == 0), stop=(cc == 1),
                    )
                v1 = sb.tile([128, C], BF16, tag="v1")
                if g % 2 == 0:
                    nc.vector.tensor_copy(v1[:], vE[:])
                else:
                    nc.scalar.copy(v1[:], vE[:])
                nc.sync.dma_start(v_dram[g * NP : (g + 1) * NP, :], v1[:])
                del H[g]

            VW = {}
            KQB = {}

            def emit_band(b):
                """column-major contiguous k/q strips for query row band b"""
                rs = b * SQR
                kst = sb.tile([128, 2, HC, WR], BF16, tag="kst",
                              name=f"kst{b}")
                nc.gpsimd.tensor_copy(
                    kst[:],
                    kqv[:, 0, :, rs : rs + WR, :].rearrange(
                        "p g r c -> p g c r"),
                )
                qst = sb.tile([128, 2, BC, SQR], BF16, tag="qst",
                              name=f"qst{b}")
                nc.gpsimd.tensor_copy(
                    qst[:],
                    kqv[:, 1, :, 2 + rs : 2 + rs + SQR,
                        2 : 2 + BC].rearrange("p g r c -> p g c r"),
                )
                KQB[b] = (kst, qst)

            def emit_P2pre(s):
                if s // NS_C not in KQB:
                    emit_band(s // NS_C)
                r0, c0 = (s // NS_C) * SQR, (s % NS_C) * SQC
                vw = vwp.tile([128, C], BF16, tag="vw", name=f"vw{s}")
                qdma = nc.sync if s % 2 == 0 else nc.gpsimd
                qdma.dma_start(
                    vw[0:NW, :], vdv[r0 : r0 + WR, c0 : c0 + WC, :]
                )
                VW[s] = vw

            def emit_P2(s):
                r0, c0 = (s // NS_C) * SQR, (s % NS_C) * SQC
                vw = VW.pop(s)
                # hw matmul operands need one contiguous free dim: read
                # from the band's column-major strips (windows = contiguous
                # column ranges there). NOTE: scores come out (c-major) — the
                # w/q index order inside the matmul is (col, row); masks and
                # vw use the same (c, r) order (host side + vw gather below).
                kst, qst = KQB[s // NS_C]
                sc_t = px2.tile([128, 512], F32, tag="scx", name=f"sc{s}")
                sc = sc_t[0:NW, 0 : NH * NQ].rearrange("p (h q) -> p h q", h=NH)
                for h in range(NH):
                    gk, mk = h // 4, h % 4
                    nc.tensor.matmul(
                        sc[:, h, :],
                        kst[32 * mk : 32 * mk + 32, gk, c0 : c0 + WC, :],
                        qst[32 * mk : 32 * mk + 32, gk, c0 : c0 + SQC, :],
                        start=True, stop=False, skip_group_check=True,
                        tile_position=(32 * mk, 0),
                    )
                    nc.tensor.matmul(
                        sc[:, h, :],
                        ident[0:NW, 0:NW],
                        masks[0:NW, s, :],
                        start=False, stop=True, skip_group_check=True,
                    )
                E = sb.tile([128, NH, NQ], BF16, tag="E")
                nc.scalar.activation(E[0:NW, :, :], sc[:], ACTF.Exp,
                                     scale=scal[0:NW, 1:2])
                cx_t = px2.tile([128, 512], F32, tag="scx", name=f"cx{s}")
                cx = cx_t[0:NQ, 0 : NH * (DK + 1)].rearrange(
                    "p (h d) -> p h d", h=NH
                )
                for h in range(NH):
                    nc.tensor.matmul(
                        cx[:, h, 0:DK],
                        E[0:NW, h, :],
                        vw[0:NW, DK * h : DK * h + DK],
                        start=True, stop=True,
                    )
                    nc.tensor.matmul(
                        cx[:, h, DK : DK + 1],
                        E[0:NW, h, :],
                        onesw[0:NW, :],
                        start=True, stop=True,
                    )
                srinv = sb.tile([NQ, NH], F32, tag="srinv")
                nc.vector.reciprocal(srinv[:], cx[:, :, DK])
                ctxn = sb.tile([64, C], BF16, tag="ctxn")
                # rows 50:64 are transpose pad (never consumed downstream);
                # memset from 32 (engines need 32-aligned start partitions),
                # the overlap is overwritten by the normalize below
                nc.gpsimd.memset(ctxn[32:64, :], 0.0)
                nc.vector.tensor_mul(
                    ctxn[0:NQ, :].rearrange("q (h d) -> q h d", h=NH),
                    cx[:, :, 0:DK],
                    srinv[:].unsqueeze(2).broadcast_to((NQ, NH, DK)),
                )
                ntpT = sb.tile([128, 2, 64], BF16, tag="ntpT")
                nc.sync.dma_start_transpose(ntpT[:], ctxn[:])
                csel = cTv[:, :, r0 : r0 + SQR, c0 : c0 + SQC]
                nc.gpsimd.tensor_copy(
                    csel, ntpT[:, :, 0:NQ].rearrange(
                        "p a (r c) -> p a r c", r=SQR
                    ),
                )

            def emit_P3(g):
                op = pe.tile([128, 512], F32, tag="ep", name=f"op{g}")[
                    0:NPO, 0:C
                ]
                for cc in range(2):
                    nc.tensor.matmul(
                        op[:],
                        cT_all[:, cc, g * NPO : g * NPO + NPO],
                        wo[:, cc, :],
                        start=(cc == 0), stop=False,
                        skip_group_check=True,
                    )
                nc.tensor.matmul(
                    op[:], ones1[0:1, 0:NPO], boe[0:1, 0:C],
                    start=False, stop=True, skip_group_check=True,
                )
                o_sb = sb.tile([NPO, C], BF16, tag="o_sb")
                if g % 2 == 0:
                    nc.scalar.copy(o_sb[:], op[:])
                else:
                    nc.vector.tensor_copy(o_sb[:], op[:])
                nc.sync.dma_start(out_d.ap()[g * NPO : (g + 1) * NPO, :], o_sb[:])

            # ---- interleaved emission schedule (software pipelining) ----
            # A1(g) -> [B(g-1)] -> A2(g), with phase-2 chunks emitted as soon
            # as their window rows are fully written, and phase-3 chunks as
            # their cT bands complete.
            p2q = list(range(NS)) if "2" in PH else []
            p3q = list(range(GO)) if "3" in PH else []
            p2_done = 0

            def p2_ready(s, g_written):
                rs = (s // NS_C) * SQR
                return (rs + WR) * HC <= g_written * NP

            def p3_ready(g3, n_p2_done):
                return (g3 // 2 + 1) * NS_C <= n_p2_done

            sched_gs = list(range(G)) if "1" in PH else []
            p2pre = list(range(NS)) if "2" in PH else []
            for gi, g in enumerate(sched_gs + [None]):
                if g is not None:
                    emit_A1(g)
                if gi >= 1 and "1" in PH:
                    emit_B1(sched_gs[gi - 1])
                if g is not None:
                    emit_A2(g)
                if gi >= 1 and "1" in PH:
                    emit_B2(sched_gs[gi - 1])
                g_written = gi if "1" in PH else G
                while p2pre and p2_ready(p2pre[0], g_written):
                    emit_P2pre(p2pre.pop(0))
                while p2q and p2q[0] in VW and (len(VW) >= 4 or not p2pre):
                    emit_P2(p2q.pop(0))
                    p2_done += 1
                    while p3q and (p3q[0] // 2 + 1) * NS_C + 3 <= p2_done:
                        emit_P3(p3q.pop(0))
            while p2pre:
                emit_P2pre(p2pre.pop(0))
            while p2q:
                emit_P2(p2q.pop(0))
                p2_done += 1
                while p3q and (p3q[0] // 2 + 1) * NS_C + 3 <= p2_done:
                    emit_P3(p3q.pop(0))
            for g3 in p3q:
                emit_P3(g3)
            if "3" not in PH:
                zz = sb.tile([NPO, C], BF16, tag="o_sb", name="zz")
                nc.vector.memset(zz[:], 0.0)
                nc.sync.dma_start(out_d.ap()[0:NPO, :], zz[:])

    nc.compile()
    return nc
